# revision 1
# baseline (speedup 1.0000x reference)
"""LSTM greedy decoder on 8 trn2 NeuronCores.

Sharding: vocab-parallel. Each core keeps a resident SBUF copy of its
4000-row W_out shard, replicates the LSTM cell, and exchanges per-step
(max, argmax) candidates via a tiny AllGather to agree on the greedy token.

Speed/precision split: the vocab projection runs in fp32r (TF32-class,
~4x faster on the PE); the greedy feedback is protected by re-evaluating
the global top-3 candidate logits exactly in fp32 (DVE dot products)
before picking the token. The LSTM recurrence itself stays exact fp32.
The x @ W_ih.T + biases term is folded into a host-precomputed table
(embed_table @ W_ih.T + b) gathered per token and added into PSUM.
"""

import numpy as np

B, H, D, V, S = 64, 512, 256, 32000, 64
NCORES = 8
VS = V // NCORES            # 4000 vocab rows per core
G4 = 4 * H                  # 2048 gate units
NT = 8                      # logits N-tiles per step
TN = VS // NT               # 500 columns per logits tile
KH = H // 128               # 4 contraction tiles over H
BIG = 1.0e9
NCAND = 3                   # exact-rechecked candidates per level
TC = 2                      # candidates shipped per logits tile


def build_program(steps=S):
    import concourse.bass as bass
    import concourse.bacc as bacc
    import concourse.mybir as mybir
    import concourse.tile as tile
    from concourse.masks import make_identity

    f32 = mybir.dt.float32
    f32r = mybir.dt.float32r
    u32 = mybir.dt.uint32
    AF = mybir.ActivationFunctionType
    OP = mybir.AluOpType
    AX = mybir.AxisListType

    nc = bacc.Bacc(num_devices=NCORES)
    h0T_p = nc.declare_dram_parameter("h0T", [H, B], f32, isOutput=False)
    c0_p = nc.declare_dram_parameter("c0", [B, H], f32, isOutput=False)
    whhT_p = nc.declare_dram_parameter("whhT", [H, G4], f32, isOutput=False)
    bias_p = nc.declare_dram_parameter("bias", [1, G4], f32, isOutput=False)
    woutT_p = nc.declare_dram_parameter("woutT", [H, VS], f32, isOutput=False)
    bout_p = nc.declare_dram_parameter("bout", [1, VS], f32, isOutput=False)
    whe_p = nc.declare_dram_parameter("whe", [V, G4], f32, isOutput=False)
    wfullb_p = nc.declare_dram_parameter("wfullb", [V, H + 1], f32, isOutput=False)
    basec_p = nc.declare_dram_parameter("basec", [B, 1], f32, isOutput=False)
    out_p = nc.declare_dram_parameter("out", [steps, B, VS], f32, isOutput=True)

    rg = [list(range(NCORES))]

    with tile.TileContext(nc) as tc:
        with (
            tc.tile_pool(name="wpool", bufs=1) as wp,
            tc.tile_pool(name="state", bufs=2) as sp,
            tc.tile_pool(name="work", bufs=2) as kp,
            tc.tile_pool(name="ps_g", bufs=1, space="PSUM") as pg,
            tc.tile_pool(name="ps_l", bufs=2, space="PSUM") as pl,
            tc.tile_pool(name="ps_t", bufs=2, space="PSUM") as pt,
            tc.tile_pool(name="dram", bufs=2, space="DRAM") as dp,
        ):
            # ---- constants (engine-local, no DMA) ----
            ident = wp.tile([128, 128], f32)
            make_identity(nc, ident[:])
            ones1 = wp.tile([1, B], f32)
            nc.vector.memset(ones1[:], 1.0)
            ones1r = wp.tile([1, B], f32r)
            nc.vector.tensor_copy(ones1r[:], ones1[:])

            # ---- resident weights (barriers cap per-inst sync-wait fan-in) ----
            tc.strict_bb_all_engine_barrier()
            whh = wp.tile([128, KH, G4], f32)
            nc.sync.dma_start(out=whh[:], in_=whhT_p[:].rearrange("(a p) n -> p a n", p=128))
            # one-shot t=0 bias lives in the same slot the per-step gx reuses
            bias = kp.tile([1, G4], f32, tag="gx", bufs=1, name="bias")
            nc.sync.dma_start(out=bias[:], in_=bias_p[:])
            tc.strict_bb_all_engine_barrier()

            # fp32r weights: stage fp32 chunks through the logits-tagged slot,
            # then round-copy (walrus requires fp32r-matmul inputs pre-rounded)
            wout = wp.tile([128, KH, VS], f32r)
            bout = wp.tile([1, VS], f32r)
            wq = woutT_p[:].rearrange("(a p) n -> p a n", p=128)
            for k in range(KH):
                stage = kp.tile([128, VS], f32, tag="logits", name=f"wstage{k}")
                nc.sync.dma_start(out=stage[:], in_=wq[:, k, :])
                nc.vector.tensor_copy(wout[:, k, :], stage[:])
                if k == 0:
                    bstage = kp.tile([1, VS], f32, tag="logits", name="bstage")
                    nc.sync.dma_start(out=bstage[:], in_=bout_p[:])
                    nc.vector.tensor_copy(bout[:], bstage[:])
                tc.strict_bb_all_engine_barrier()

            basec = wp.tile([B, 1], f32)
            nc.sync.dma_start(out=basec[:], in_=basec_p[:])
            tbase = wp.tile([B, NT * NCAND], f32)  # col 3n+r -> n*TN
            for n in range(NT):
                nc.vector.memset(tbase[:, NCAND * n : NCAND * (n + 1)], float(n * TN))
            hT = sp.tile([128, KH, B], f32, tag="hT")
            nc.sync.dma_start(out=hT[:], in_=h0T_p[:].rearrange("(a p) b -> p a b", p=128))
            tc.strict_bb_all_engine_barrier()
            hTr = sp.tile([128, KH, B], f32r, tag="hTr")
            nc.vector.tensor_copy(hTr[:], hT[:])
            c_prev = sp.tile([B, H], f32, tag="c")
            nc.sync.dma_start(out=c_prev[:], in_=c0_p[:])
            tc.strict_bb_all_engine_barrier()

            def select_topk(vals, idxs, m8, tag):
                """Top-NCAND (value desc, index asc) with distinct indices.
                vals/idxs: [B, ...] f32 APs; m8: [B, 8] sorted maxes of vals.
                Returns ([B, NCAND] vals, [B, NCAND] idxs)."""
                shape = list(vals.shape)
                red_ax = {2: AX.X, 3: AX.XY, 4: AX.XYZ}[len(shape)]
                sv = kp.tile([B, NCAND], f32, tag=f"sv_{tag}", bufs=1, name=f"sv{tag}")
                si = kp.tile([B, NCAND], f32, tag=f"si_{tag}", bufs=1, name=f"si{tag}")
                nc.vector.tensor_copy(sv[:], m8[:, 0:NCAND])
                mask = kp.tile(shape, f32, tag=f"mk_{tag}", bufs=1, name=f"mk{tag}")
                ne = kp.tile(shape, f32, tag=f"ne_{tag}", bufs=1, name=f"ne{tag}")
                cand = kp.tile(shape, f32, tag=f"cd_{tag}", bufs=1, name=f"cd{tag}")
                for k in range(NCAND):
                    nc.vector.tensor_scalar(mask[:], vals[:], m8[:, k : k + 1], None, op0=OP.is_ge)
                    for j in range(k):
                        nc.vector.tensor_scalar(ne[:], idxs[:], si[:, j : j + 1], None, op0=OP.not_equal)
                        nc.vector.tensor_tensor(mask[:], mask[:], ne[:], op=OP.mult)
                    nc.vector.tensor_scalar(cand[:], mask[:], -BIG, BIG, op0=OP.mult, op1=OP.add)
                    nc.vector.tensor_tensor(cand[:], idxs[:], cand[:], op=OP.add)
                    nc.vector.tensor_reduce(si[:, k : k + 1], cand[:], axis=red_ax, op=OP.min)
                return sv, si

            gx = None  # gathered (x @ W_ih.T + b) rows, [B, G4]
            for t in range(steps):
                # ---- gates: h-part on PE (exact fp32); x-part DVE-added ----
                gates = pg.tile([B, G4], f32, tag="gates")
                for n in range(4):
                    ns = slice(n * 512, (n + 1) * 512)
                    if gx is None:  # t == 0: x is zero, init with biases
                        nc.tensor.matmul(out=gates[:, ns], lhsT=ones1[:], rhs=bias[:, ns],
                                         start=True, stop=False)
                    for k in range(KH):
                        nc.tensor.matmul(out=gates[:, ns], lhsT=hT[:, k, :], rhs=whh[:, k, ns],
                                         start=(gx is not None and k == 0), stop=(k == KH - 1))
                if gx is not None:
                    for n in range(4):
                        ns = slice(n * 512, (n + 1) * 512)
                        nc.vector.tensor_tensor(gates[:, ns], gates[:, ns], gx[:, ns], op=OP.add)

                # ---- LSTM pointwise (gate order i, f, g, o) ----
                si_t = kp.tile([B, H], f32, tag="si", bufs=1)
                sf = kp.tile([B, H], f32, tag="sf", bufs=1)
                tg = kp.tile([B, H], f32, tag="tg", bufs=1)
                so = kp.tile([B, H], f32, tag="so", bufs=1)
                nc.scalar.activation(si_t[:], gates[:, 0:512], AF.Sigmoid)
                nc.scalar.activation(sf[:], gates[:, 512:1024], AF.Sigmoid)
                nc.scalar.activation(tg[:], gates[:, 1024:1536], AF.Tanh)
                nc.scalar.activation(so[:], gates[:, 1536:2048], AF.Sigmoid)
                fc = kp.tile([B, H], f32, tag="fc", bufs=1)
                nc.vector.tensor_tensor(fc[:], sf[:], c_prev[:], op=OP.mult)
                ig = kp.tile([B, H], f32, tag="ig", bufs=1)
                nc.vector.tensor_tensor(ig[:], si_t[:], tg[:], op=OP.mult)
                c_new = sp.tile([B, H], f32, tag="c")
                nc.vector.tensor_tensor(c_new[:], fc[:], ig[:], op=OP.add)
                tct = kp.tile([B, H], f32, tag="tct", bufs=1)
                nc.scalar.activation(tct[:], c_new[:], AF.Tanh)
                h1a = kp.tile([B, H + 1], f32, tag="h1")
                h1 = h1a[:, 0:H]
                nc.vector.tensor_tensor(h1[:], so[:], tct[:], op=OP.mult)
                nc.vector.memset(h1a[:, H : H + 1], 1.0)
                c_prev = c_new

                # ---- h1 -> h1.T tiles (PE transpose); fp32 + fp32r copies ----
                hT = sp.tile([128, KH, B], f32, tag="hT")
                hTr = sp.tile([128, KH, B], f32r, tag="hTr")
                for k in range(KH):
                    tp = pt.tile([128, B], f32, tag="tp")
                    nc.tensor.transpose(out=tp[:], in_=h1[:, k * 128 : (k + 1) * 128],
                                        identity=ident[0:B, 0:B])
                    nc.vector.tensor_copy(hT[:, k, :], tp[:])
                    nc.vector.tensor_copy(hTr[:, k, :], tp[:])

                # ---- logits shard in fp32r + per-tile top-3 ----
                logits = kp.tile([B, VS], f32, tag="logits")
                tv = kp.tile([B, NT * NCAND], f32, tag="tv")
                ti = kp.tile([B, NT * NCAND], f32, tag="ti")
                m8 = kp.tile([B, 8], f32, tag="m8", bufs=1)
                i8 = kp.tile([B, 8], u32, tag="i8", bufs=1)
                for n in range(NT):
                    ns = slice(n * TN, (n + 1) * TN)
                    lg = pl.tile([B, TN], f32, tag="lg")
                    nc.tensor.matmul(out=lg[:], lhsT=ones1r[:], rhs=bout[:, ns],
                                     start=True, stop=False)
                    for k in range(KH):
                        nc.tensor.matmul(out=lg[:], lhsT=hTr[:, k, :], rhs=wout[:, k, ns],
                                         start=False, stop=(k == KH - 1))
                    nc.scalar.activation(logits[:, ns], lg[:], AF.Copy)
                    if t < steps - 1:
                        cs = slice(NCAND * n, NCAND * (n + 1))
                        nc.vector.max(out=m8[:], in_=logits[:, ns])
                        nc.vector.tensor_copy(tv[:, cs], m8[:, 0:NCAND])
                        nc.vector.max_index(out=i8[:], in_max=m8[:], in_values=logits[:, ns])
                        nc.vector.tensor_copy(ti[:, cs], i8[:, 0:NCAND])

                nc.sync.dma_start(out=out_p[t], in_=logits[:])
                if t == steps - 1:
                    break

                # ---- local top-3 (distinct indices), global vocab ids ----
                nc.vector.tensor_tensor(ti[:], ti[:], tbase[:], op=OP.add)
                lm8 = kp.tile([B, 8], f32, tag="lm8", bufs=1)
                nc.vector.max(out=lm8[:], in_=tv[:])
                lv, li = select_topk(tv[:], ti[:], lm8, "l")
                nc.vector.tensor_scalar(li[:], li[:], basec[:, 0:1], None, op0=OP.add)

                # ---- single AllGather of (3 vals | 3 ids) per core ----
                agin = dp.tile([2 * NCAND, B], f32, tag="agin")
                nc.sync.dma_start(out=agin[0:NCAND, :].rearrange("k b -> b k"), in_=lv[:])
                nc.sync.dma_start(out=agin[NCAND : 2 * NCAND, :].rearrange("k b -> b k"), in_=li[:])
                agout = dp.tile([NCORES * 2 * NCAND, B], f32, tag="agout", addr_space="Shared")
                nc.gpsimd.collective_compute(
                    "AllGather", OP.bypass, replica_groups=rg,
                    ins=[agin[:].opt()], outs=[agout[:].opt()],
                )
                gall = kp.tile([B, NCORES, 2 * NCAND], f32, tag="gall")
                ago = agout[:].rearrange("(r s) b -> b r s", s=2 * NCAND)
                nc.sync.dma_start(out=gall[:], in_=ago[:])
                gv = gall[:, :, 0:NCAND]
                gi = gall[:, :, NCAND : 2 * NCAND]

                # ---- global top-3 candidates ----
                gm8 = kp.tile([B, 8], f32, tag="gm8", bufs=1)
                nc.vector.max(out=gm8[:], in_=gv)
                _, gci = select_topk(gv, gi, gm8, "g")

                # ---- exact fp32 recheck of the 3 candidates ----
                ev = kp.tile([B, 8], f32, tag="ev", bufs=1)
                nc.vector.memset(ev[:], -BIG)
                cu = kp.tile([B, NCAND], u32, tag="cu", bufs=1)
                nc.vector.tensor_copy(cu[:], gci[:])
                wrow = kp.tile([B, H + 1], f32, tag="wrow", bufs=2)
                prod = kp.tile([B, H + 1], f32, tag="prod", bufs=1)
                for k in range(NCAND):
                    nc.gpsimd.indirect_dma_start(
                        out=wrow[:], out_offset=None, in_=wfullb_p[:],
                        in_offset=bass.IndirectOffsetOnAxis(ap=cu[:, k : k + 1], axis=0),
                    )
                    nc.vector.tensor_tensor(prod[:], h1a[:], wrow[:], op=OP.mult)
                    nc.vector.tensor_reduce(ev[:, k : k + 1], prod[:], axis=AX.X, op=OP.add)

                em8 = kp.tile([B, 8], f32, tag="em8", bufs=1)
                nc.vector.max(out=em8[:], in_=ev[:])
                emask = kp.tile([B, NCAND], f32, tag="emask", bufs=1)
                nc.vector.tensor_scalar(emask[:], ev[:, 0:NCAND], em8[:, 0:1], None, op0=OP.is_ge)
                nc.vector.tensor_scalar(emask[:], emask[:], -BIG, BIG, op0=OP.mult, op1=OP.add)
                nc.vector.tensor_tensor(emask[:], gci[:], emask[:], op=OP.add)
                gidxf = kp.tile([B, 1], f32, tag="gidxf", bufs=1)
                nc.vector.tensor_reduce(gidxf[:], emask[:], axis=AX.X, op=OP.min)
                gidx = kp.tile([B, 1], u32, tag="gidx")
                nc.vector.tensor_copy(gidx[:], gidxf[:])

                # ---- gather the token's precomputed gate row ----
                gx = kp.tile([B, G4], f32, tag="gx", bufs=1)
                nc.gpsimd.indirect_dma_start(
                    out=gx[:], out_offset=None, in_=whe_p[:],
                    in_offset=bass.IndirectOffsetOnAxis(ap=gidx[:, :1], axis=0),
                )

    nc.finalize()  # Bacc: runs compile() legalization passes
    return nc


def make_in_maps(inputs):
    inp = {k: np.asarray(v) for k, v in inputs.items()}
    h0 = inp["h0"].astype(np.float32)
    c0 = inp["c0"].astype(np.float32)
    W_ih = inp["W_ih"].astype(np.float32)
    W_hh = inp["W_hh"].astype(np.float32)
    b = (inp["b_ih"].astype(np.float32) + inp["b_hh"].astype(np.float32)).reshape(1, G4)
    W_out = inp["W_out"].astype(np.float32)
    b_out = inp["b_out"].astype(np.float32)
    emb = inp["embed_table"].astype(np.float32)
    # x @ W_ih.T + b for every vocab row, fp32
    whe = (emb @ W_ih.T + b).astype(np.float32)
    wfullb = np.ascontiguousarray(
        np.concatenate([W_out, b_out.reshape(V, 1)], axis=1).astype(np.float32))
    in_maps = []
    for c in range(NCORES):
        base = c * VS
        in_maps.append({
            "h0T": np.ascontiguousarray(h0.T),
            "c0": np.ascontiguousarray(c0),
            "whhT": np.ascontiguousarray(W_hh.T),
            "bias": b,
            "woutT": np.ascontiguousarray(W_out[base : base + VS].T),
            "bout": np.ascontiguousarray(b_out[base : base + VS].reshape(1, VS)),
            "whe": whe,
            "wfullb": wfullb,
            "basec": np.full((B, 1), float(base), np.float32),
        })
    return in_maps


def run(inputs, steps=S, trace=False):
    from concourse.bass_utils import run_bass_kernel_spmd

    nc = build_program(steps)
    res = run_bass_kernel_spmd(nc, make_in_maps(inputs), list(range(NCORES)),
                               trace=trace)
    outs = [res.results[c]["out"] for c in range(NCORES)]      # each [steps, B, VS]
    full = np.concatenate(outs, axis=2)                        # [steps, B, V]
    return np.ascontiguousarray(np.transpose(full, (1, 0, 2))), res


def kernel(**inputs):
    out, _ = run(inputs, steps=S, trace=False)
    return out.astype(np.float32)



# revision 7
# speedup vs baseline: 1.1887x; 1.1887x over previous
"""LSTM greedy decoder on 8 trn2 NeuronCores.

Vocab-parallel: each core keeps a resident fp32r SBUF copy of its
4000-row W_out shard, replicates the LSTM cell (exact fp32), and agrees
on the greedy token by exchanging per-core top-3 candidates via a tiny
AllGather.

Layout/scheduling notes:
- the per-core top-3 candidate logits are re-evaluated exactly in fp32
  BEFORE the AllGather (one 3-row indirect gather + fused mult+reduce
  dots), so the exchanged values are exact and the global step is a
  plain max — no post-collective recheck round-trip.
- candidate finding is one MAX8 + MAX_INDEX8 over the whole [B, 4000]
  shard (true top-8, sorted), not a per-tile cascade.
- AllGather staging uses DVE 32x32 stream-transposes + contiguous DMAs
  (the naive layout generates thousands of 4-byte descriptors).
- gates matmuls for step t+1 are emitted before the exchange tail so
  the PE works under the collective; no PE instructions appear in the
  tail (PE queues are FIFO — a late transpose would block those gates).
- gate layout is host-permuted to [i, f, o, g] so one fused Sigmoid
  covers i/f/o; the x-contribution table whe = embed @ W_ih.T + b is
  gathered per token and added in a single [B, 4H] op.
"""

import numpy as np

B, H, D, V, S = 64, 512, 256, 32000, 64
NCORES = 8
VS = V // NCORES            # 4000 vocab rows per core
G4 = 4 * H                  # 2048 gate units
NT = 8                      # logits N-tiles per step
TN = VS // NT               # 500 columns per logits tile
KH = H // 128               # 4 contraction tiles over H
BIG = 1.0e9
NCAND = 3                   # exact-rechecked candidates per core


def build_program(steps=S):
    import concourse.bass as bass
    import concourse.bacc as bacc
    import concourse.mybir as mybir
    import concourse.tile as tile
    from concourse.masks import make_identity

    f32 = mybir.dt.float32
    f32r = mybir.dt.float32r
    u32 = mybir.dt.uint32
    AF = mybir.ActivationFunctionType
    OP = mybir.AluOpType
    AX = mybir.AxisListType

    nc = bacc.Bacc(num_devices=NCORES)
    h0T_p = nc.declare_dram_parameter("h0T", [H, B], f32, isOutput=False)
    c0_p = nc.declare_dram_parameter("c0", [B, H], f32, isOutput=False)
    whhT_p = nc.declare_dram_parameter("whhT", [H, G4], f32, isOutput=False)
    bias_p = nc.declare_dram_parameter("bias", [1, G4], f32, isOutput=False)
    woutT_p = nc.declare_dram_parameter("woutT", [H, VS], f32, isOutput=False)
    bout_p = nc.declare_dram_parameter("bout", [1, VS], f32, isOutput=False)
    whe_p = nc.declare_dram_parameter("whe", [V, G4], f32, isOutput=False)
    wfullb_p = nc.declare_dram_parameter("wfullb", [V, H + 1], f32, isOutput=False)
    basec_p = nc.declare_dram_parameter("basec", [B, 1], f32, isOutput=False)
    out_p = nc.declare_dram_parameter("out", [steps, B, VS], f32, isOutput=True)

    rg = [list(range(NCORES))]

    with tile.TileContext(nc) as tc:
        with (
            tc.tile_pool(name="wpool", bufs=1) as wp,
            tc.tile_pool(name="state", bufs=2) as sp,
            tc.tile_pool(name="work", bufs=2) as kp,
            tc.tile_pool(name="ps_g", bufs=1, space="PSUM") as pg,
            tc.tile_pool(name="ps_l", bufs=2, space="PSUM") as pl,
            tc.tile_pool(name="ps_t", bufs=2, space="PSUM") as pt,
            tc.tile_pool(name="dram", bufs=2, space="DRAM") as dp,
        ):
            # ---- constants (engine-local, no DMA) ----
            ident = wp.tile([128, 128], f32)
            make_identity(nc, ident[:])
            ones1 = wp.tile([1, B], f32)
            nc.vector.memset(ones1[:], 1.0)
            ones1r = wp.tile([1, B], f32r)
            nc.vector.tensor_copy(ones1r[:], ones1[:])

            # ---- resident weights (barriers cap per-inst sync-wait fan-in) ----
            tc.strict_bb_all_engine_barrier()
            whh = wp.tile([128, KH, G4], f32)
            nc.sync.dma_start(out=whh[:], in_=whhT_p[:].rearrange("(a p) n -> p a n", p=128))
            # one-shot t=0 bias lives in the same slot the per-step gx reuses
            bias = kp.tile([1, G4], f32, tag="gx", bufs=1, name="bias")
            nc.sync.dma_start(out=bias[:], in_=bias_p[:])
            basec = wp.tile([B, 1], f32)
            nc.sync.dma_start(out=basec[:], in_=basec_p[:])
            tc.strict_bb_all_engine_barrier()

            # fp32r weights: stage fp32 chunks through the logits-tagged slot,
            # then round-copy (walrus requires fp32r-matmul inputs pre-rounded)
            wout = wp.tile([128, KH, VS], f32r)
            bout = wp.tile([1, VS], f32r)
            wq = woutT_p[:].rearrange("(a p) n -> p a n", p=128)
            for k in range(KH):
                stage = kp.tile([128, VS], f32, tag="logits", name=f"wstage{k}")
                nc.sync.dma_start(out=stage[:], in_=wq[:, k, :])
                nc.vector.tensor_copy(wout[:, k, :], stage[:])
                if k == 0:
                    bstage = kp.tile([1, VS], f32, tag="logits", name="bstage")
                    nc.sync.dma_start(out=bstage[:], in_=bout_p[:])
                    nc.vector.tensor_copy(bout[:], bstage[:])
                tc.strict_bb_all_engine_barrier()

            # exchange staging tiles (allocated once; padding defined once)
            pk = wp.tile([B, 32], f32)          # [ev0..2 | id0..2 | pad]
            nc.vector.memset(pk[:], 0.0)
            pkT = wp.tile([B, 32], f32)
            gpre = wp.tile([B, B], f32)
            nc.vector.memset(gpre[:], 0.0)
            gpT = wp.tile([B, B], f32)

            hT = sp.tile([128, KH, B], f32, tag="hT")
            nc.sync.dma_start(out=hT[:], in_=h0T_p[:].rearrange("(a p) b -> p a b", p=128))
            tc.strict_bb_all_engine_barrier()
            hTr = sp.tile([128, KH, B], f32r, tag="hTr")
            nc.vector.tensor_copy(hTr[:], hT[:])
            c_prev = sp.tile([B, H], f32, tag="c")
            nc.sync.dma_start(out=c_prev[:], in_=c0_p[:])
            tc.strict_bb_all_engine_barrier()

            # ---- gates for t=0: bias + W_hh @ h0 (exact fp32) ----
            gates = pg.tile([B, G4], f32, tag="gates")
            for n in range(4):
                ns = slice(n * 512, (n + 1) * 512)
                nc.tensor.matmul(out=gates[:, ns], lhsT=ones1[:], rhs=bias[:, ns],
                                 start=True, stop=False)
                for k in range(KH):
                    nc.tensor.matmul(out=gates[:, ns], lhsT=hT[:, k, :], rhs=whh[:, k, ns],
                                     start=False, stop=(k == KH - 1))

            gx = None
            for t in range(steps):
                # ---- LSTM pointwise; gate layout is [i, f, o, g] ----
                gsb = kp.tile([B, G4], f32, tag="gsb", bufs=1)
                if gx is None:
                    nc.scalar.activation(gsb[:], gates[:], AF.Copy)
                else:
                    nc.vector.tensor_tensor(gsb[:], gates[:], gx[:], op=OP.add)
                acts = kp.tile([B, G4], f32, tag="acts", bufs=1)
                nc.scalar.activation(acts[:, 0 : 3 * H], gsb[:, 0 : 3 * H], AF.Sigmoid)
                nc.scalar.activation(acts[:, 3 * H : G4], gsb[:, 3 * H : G4], AF.Tanh)
                fc = kp.tile([B, H], f32, tag="fc", bufs=1)
                nc.vector.tensor_tensor(fc[:], acts[:, H : 2 * H], c_prev[:], op=OP.mult)
                ig = kp.tile([B, H], f32, tag="ig", bufs=1)
                nc.vector.tensor_tensor(ig[:], acts[:, 0:H], acts[:, 3 * H : G4], op=OP.mult)
                c_new = sp.tile([B, H], f32, tag="c")
                nc.vector.tensor_tensor(c_new[:], fc[:], ig[:], op=OP.add)
                tct = kp.tile([B, H], f32, tag="tct", bufs=1)
                nc.scalar.activation(tct[:], c_new[:], AF.Tanh)
                h1a = kp.tile([B, H + 1], f32, tag="h1")
                h1 = h1a[:, 0:H]
                nc.vector.tensor_tensor(h1[:], acts[:, 2 * H : 3 * H], tct[:], op=OP.mult)
                nc.vector.memset(h1a[:, H : H + 1], 1.0)
                c_prev = c_new

                # ---- h1 -> h1.T tiles (PE transpose); fp32 on ACT, fp32r on DVE ----
                hT = sp.tile([128, KH, B], f32, tag="hT")
                hTr = sp.tile([128, KH, B], f32r, tag="hTr")
                for k in range(KH):
                    tp = pt.tile([128, B], f32, tag="tp")
                    nc.tensor.transpose(out=tp[:], in_=h1[:, k * 128 : (k + 1) * 128],
                                        identity=ident[0:B, 0:B])
                    nc.scalar.activation(hT[:, k, :], tp[:], AF.Copy)
                    nc.vector.tensor_copy(hTr[:, k, :], tp[:])

                # ---- logits shard in fp32r ----
                logits = kp.tile([B, VS], f32, tag="logits")
                for n in range(NT):
                    ns = slice(n * TN, (n + 1) * TN)
                    lg = pl.tile([B, TN], f32, tag="lg")
                    nc.tensor.matmul(out=lg[:], lhsT=ones1r[:], rhs=bout[:, ns],
                                     start=True, stop=False)
                    for k in range(KH):
                        nc.tensor.matmul(out=lg[:], lhsT=hTr[:, k, :], rhs=wout[:, k, ns],
                                         start=False, stop=(k == KH - 1))
                    nc.scalar.activation(logits[:, ns], lg[:], AF.Copy)
                nc.scalar.dma_start(out=out_p[t], in_=logits[:])
                if t == steps - 1:
                    break

                # ---- gates h-part for t+1 (exact fp32) — PE works under the tail ----
                gates = pg.tile([B, G4], f32, tag="gates")
                for n in range(4):
                    ns = slice(n * 512, (n + 1) * 512)
                    for k in range(KH):
                        nc.tensor.matmul(out=gates[:, ns], lhsT=hT[:, k, :], rhs=whh[:, k, ns],
                                         start=(k == 0), stop=(k == KH - 1))

                # ---- local top-3 (true top-8 of the shard, sorted) ----
                m8 = kp.tile([B, 8], f32, tag="m8", bufs=1)
                i8 = kp.tile([B, 8], u32, tag="i8", bufs=1)
                nc.vector.max(out=m8[:], in_=logits[:])
                nc.vector.max_index(out=i8[:], in_max=m8[:], in_values=logits[:])
                tif = kp.tile([B, NCAND], f32, tag="tif", bufs=1)
                nc.vector.tensor_copy(tif[:], i8[:, 0:NCAND])
                nc.vector.tensor_scalar(pk[:, NCAND : 2 * NCAND], tif[:], basec[:, 0:1], None, op0=OP.add)
                cu = kp.tile([B, NCAND], u32, tag="cu", bufs=1)
                nc.vector.tensor_copy(cu[:], pk[:, NCAND : 2 * NCAND])

                # ---- exact fp32 recheck of the 3 local candidates ----
                # (one indirect per row: multi-offset indirects and
                # tensor_tensor_reduce both mislower on hardware)
                wrow3 = kp.tile([B, NCAND, H + 1], f32, tag="wrow3", bufs=1)
                for j in range(NCAND):
                    nc.gpsimd.indirect_dma_start(
                        out=wrow3[:, j], out_offset=None, in_=wfullb_p[:],
                        in_offset=bass.IndirectOffsetOnAxis(ap=cu[:, j : j + 1], axis=0),
                    )
                prod3 = kp.tile([B, NCAND, H + 1], f32, tag="prod3", bufs=1)
                for j in range(NCAND):
                    nc.vector.tensor_tensor(prod3[:, j], wrow3[:, j], h1a[:], op=OP.mult)
                nc.vector.tensor_reduce(pk[:, 0:NCAND], prod3[:], axis=AX.X, op=OP.add)

                # ---- AllGather of (3 exact vals | 3 ids) per core ----
                nc.vector.transpose(pkT[:], pk[:])
                agin = dp.tile([2 * NCAND, B], f32, tag="agin")
                nc.sync.dma_start(out=agin[:, 0:32], in_=pkT[0 : 2 * NCAND, 0:32])
                nc.sync.dma_start(out=agin[:, 32:64], in_=pkT[32 : 32 + 2 * NCAND, 0:32])
                agout = dp.tile([NCORES * 2 * NCAND, B], f32, tag="agout", addr_space="Shared")
                nc.gpsimd.collective_compute(
                    "AllGather", OP.bypass, replica_groups=rg,
                    ins=[agin[:].opt()], outs=[agout[:].opt()],
                )
                # block-swapped readback so one DVE stream-transpose finishes it
                nc.sync.dma_start(out=gpre[0:32, 0:32], in_=agout[0:32, 0:32])
                nc.sync.dma_start(out=gpre[0:16, 32:64], in_=agout[32:48, 0:32])
                nc.sync.dma_start(out=gpre[32:64, 0:32], in_=agout[0:32, 32:64])
                nc.sync.dma_start(out=gpre[32:48, 32:64], in_=agout[32:48, 32:64])
                nc.vector.transpose(gpT[:], gpre[:])
                gall = gpT[:, 0:48].rearrange("b (r s) -> b r s", s=2 * NCAND)
                gv = gall[:, :, 0:NCAND]
                gi = gall[:, :, NCAND : 2 * NCAND]

                # ---- global argmax over 24 exact candidates (min-id tiebreak) ----
                gm8 = kp.tile([B, 8], f32, tag="gm8", bufs=1)
                nc.vector.max(out=gm8[:], in_=gv)
                msk = kp.tile([B, NCORES, NCAND], f32, tag="msk", bufs=1)
                nc.vector.tensor_scalar(msk[:], gv, gm8[:, 0:1], None, op0=OP.is_ge)
                nc.vector.tensor_scalar(msk[:], msk[:], -BIG, BIG, op0=OP.mult, op1=OP.add)
                nc.vector.tensor_tensor(msk[:], gi, msk[:], op=OP.add)
                gidxf = kp.tile([B, 1], f32, tag="gidxf", bufs=1)
                nc.vector.tensor_reduce(gidxf[:], msk[:], axis=AX.XY, op=OP.min)
                gidx = kp.tile([B, 1], u32, tag="gidx")
                nc.vector.tensor_copy(gidx[:], gidxf[:])

                # ---- gather the token's precomputed gate row ----
                gx = kp.tile([B, G4], f32, tag="gx", bufs=1)
                nc.gpsimd.indirect_dma_start(
                    out=gx[:], out_offset=None, in_=whe_p[:],
                    in_offset=bass.IndirectOffsetOnAxis(ap=gidx[:, :1], axis=0),
                )

    nc.finalize()  # Bacc: runs compile() legalization passes
    return nc


# gate-unit permutation [i, f, o, g] (torch order in the weights is i, f, g, o)
_PERM = np.concatenate([np.arange(0, 1024), np.arange(1536, 2048), np.arange(1024, 1536)])


def make_in_maps(inputs):
    inp = {k: np.asarray(v) for k, v in inputs.items()}
    h0 = inp["h0"].astype(np.float32)
    c0 = inp["c0"].astype(np.float32)
    W_ih = inp["W_ih"].astype(np.float32)
    W_hh = inp["W_hh"].astype(np.float32)
    b = (inp["b_ih"].astype(np.float32) + inp["b_hh"].astype(np.float32))
    W_out = inp["W_out"].astype(np.float32)
    b_out = inp["b_out"].astype(np.float32)
    emb = inp["embed_table"].astype(np.float32)
    # x @ W_ih.T + b for every vocab row, fp32, gate units permuted to [i,f,o,g]
    whe = np.ascontiguousarray((emb @ W_ih.T + b)[:, _PERM].astype(np.float32))
    wfullb = np.ascontiguousarray(
        np.concatenate([W_out, b_out.reshape(V, 1)], axis=1).astype(np.float32))
    whhT = np.ascontiguousarray(W_hh[_PERM].T)
    bias = np.ascontiguousarray(b[_PERM].reshape(1, G4))
    in_maps = []
    for c in range(NCORES):
        base = c * VS
        in_maps.append({
            "h0T": np.ascontiguousarray(h0.T),
            "c0": np.ascontiguousarray(c0),
            "whhT": whhT,
            "bias": bias,
            "woutT": np.ascontiguousarray(W_out[base : base + VS].T),
            "bout": np.ascontiguousarray(b_out[base : base + VS].reshape(1, VS)),
            "whe": whe,
            "wfullb": wfullb,
            "basec": np.full((B, 1), float(base), np.float32),
        })
    return in_maps


def run(inputs, steps=S, trace=False):
    from concourse.bass_utils import run_bass_kernel_spmd

    nc = build_program(steps)
    res = run_bass_kernel_spmd(nc, make_in_maps(inputs), list(range(NCORES)),
                               trace=trace)
    outs = [res.results[c]["out"] for c in range(NCORES)]      # each [steps, B, VS]
    full = np.concatenate(outs, axis=2)                        # [steps, B, V]
    return np.ascontiguousarray(np.transpose(full, (1, 0, 2))), res


def kernel(**inputs):
    out, _ = run(inputs, steps=S, trace=False)
    return out.astype(np.float32)


# revision 10
# speedup vs baseline: 1.1920x; 1.0028x over previous
"""LSTM greedy decoder on 8 trn2 NeuronCores.

Vocab-parallel: each core keeps a resident fp32r SBUF copy of its
4000-row W_out shard, replicates the LSTM cell (exact fp32), and agrees
on the greedy token by exchanging per-core top-3 candidates via a tiny
AllGather.

Scheduling notes:
- per-tile MAX8/MAX_INDEX8 (true sorted top-8) run pipelined under the
  logits matmuls; a short mask dance then picks the core's top-3
  distinct candidates from the pooled per-tile top-3s.
- the top-3 candidate logits are re-evaluated exactly in fp32 BEFORE
  the AllGather (indirect row gathers + fp32 dots), so the exchanged
  values are exact and the global step is a plain max.
- AllGather staging uses DVE 32x32 stream-transposes + contiguous DMAs
  (a naive layout generates thousands of 4-byte descriptors).
- gates matmuls for step t+1 are emitted before the exchange tail so
  the PE works under the collective; no PE instructions appear in the
  tail (PE queues are FIFO — a late transpose would block those gates).
- all single-buffer tail scratch tiles are allocated once, outside the
  step loop: per-iteration bufs=1 re-allocation hits the tile
  framework's min-join fallback, which stalls the allocation until
  every engine (including the ~30us-behind Tensor queue) passes the
  previous release.
- gate layout is host-permuted to [i, f, o, g] so one fused Sigmoid
  covers i/f/o; the x-contribution table whe = embed @ W_ih.T + b is
  gathered per token and added in a single [B, 4H] op.
"""

import numpy as np

B, H, D, V, S = 64, 512, 256, 32000, 64
NCORES = 8
VS = V // NCORES            # 4000 vocab rows per core
G4 = 4 * H                  # 2048 gate units
NT = 8                      # logits N-tiles per step
TN = VS // NT               # 500 columns per logits tile
KH = H // 128               # 4 contraction tiles over H
BIG = 1.0e9
NCAND = 3                   # exact-rechecked candidates per core


def build_program(steps=S):
    import concourse.bass as bass
    import concourse.bacc as bacc
    import concourse.mybir as mybir
    import concourse.tile as tile
    from concourse.masks import make_identity

    f32 = mybir.dt.float32
    f32r = mybir.dt.float32r
    u32 = mybir.dt.uint32
    AF = mybir.ActivationFunctionType
    OP = mybir.AluOpType
    AX = mybir.AxisListType

    nc = bacc.Bacc(num_devices=NCORES)
    h0T_p = nc.declare_dram_parameter("h0T", [H, B], f32, isOutput=False)
    c0_p = nc.declare_dram_parameter("c0", [B, H], f32, isOutput=False)
    whhT_p = nc.declare_dram_parameter("whhT", [H, G4], f32, isOutput=False)
    bias_p = nc.declare_dram_parameter("bias", [1, G4], f32, isOutput=False)
    woutT_p = nc.declare_dram_parameter("woutT", [H, VS], f32, isOutput=False)
    bout_p = nc.declare_dram_parameter("bout", [1, VS], f32, isOutput=False)
    whe_p = nc.declare_dram_parameter("whe", [V, G4], f32, isOutput=False)
    wfullb_p = nc.declare_dram_parameter("wfullb", [V, H + 1], f32, isOutput=False)
    tbase_p = nc.declare_dram_parameter("tbase", [B, NT * NCAND], f32, isOutput=False)
    out_p = nc.declare_dram_parameter("out", [steps, B, VS], f32, isOutput=True)

    rg = [list(range(NCORES))]

    with tile.TileContext(nc) as tc:
        with (
            tc.tile_pool(name="wpool", bufs=1) as wp,
            tc.tile_pool(name="state", bufs=2) as sp,
            tc.tile_pool(name="work", bufs=2) as kp,
            tc.tile_pool(name="ps_g", bufs=1, space="PSUM") as pg,
            tc.tile_pool(name="ps_l", bufs=2, space="PSUM") as pl,
            tc.tile_pool(name="ps_t", bufs=2, space="PSUM") as pt,
            tc.tile_pool(name="dram", bufs=2, space="DRAM") as dp,
        ):
            # ---- constants (engine-local, no DMA) ----
            ident = wp.tile([128, 128], f32)
            make_identity(nc, ident[:])
            ones1 = wp.tile([1, B], f32)
            nc.vector.memset(ones1[:], 1.0)
            ones1r = wp.tile([1, B], f32r)
            nc.vector.tensor_copy(ones1r[:], ones1[:])

            # ---- resident weights (barriers cap per-inst sync-wait fan-in) ----
            tc.strict_bb_all_engine_barrier()
            whh = wp.tile([128, KH, G4], f32)
            nc.sync.dma_start(out=whh[:], in_=whhT_p[:].rearrange("(a p) n -> p a n", p=128))
            bias = wp.tile([1, G4], f32)
            nc.sync.dma_start(out=bias[:], in_=bias_p[:])
            tbase = wp.tile([B, NT * NCAND], f32)
            nc.sync.dma_start(out=tbase[:], in_=tbase_p[:])
            tc.strict_bb_all_engine_barrier()

            # fp32r weights: stage fp32 chunks through the logits-tagged slot,
            # then round-copy (walrus requires fp32r-matmul inputs pre-rounded)
            wout = wp.tile([128, KH, VS], f32r)
            bout = wp.tile([1, VS], f32r)
            wq = woutT_p[:].rearrange("(a p) n -> p a n", p=128)
            for k in range(KH):
                stage = kp.tile([128, VS], f32, tag="logits", name=f"wstage{k}")
                nc.sync.dma_start(out=stage[:], in_=wq[:, k, :])
                nc.vector.tensor_copy(wout[:, k, :], stage[:])
                if k == 0:
                    bstage = kp.tile([1, VS], f32, tag="logits", name="bstage")
                    nc.sync.dma_start(out=bstage[:], in_=bout_p[:])
                    nc.vector.tensor_copy(bout[:], bstage[:])
                tc.strict_bb_all_engine_barrier()

            # ---- loop-invariant scratch (alloc once: per-iter bufs=1
            # re-allocs hit the min-join fallback and stall on Tensor) ----
            pk = wp.tile([B, 32], f32)          # [ev0..2 | id0..2 | pad]
            nc.vector.memset(pk[:], 0.0)
            pkT = wp.tile([B, 32], f32)
            gpre = wp.tile([B, B], f32)
            nc.vector.memset(gpre[:], 0.0)
            gpT = wp.tile([B, B], f32)
            tv = wp.tile([B, NT, NCAND], f32)
            ti = wp.tile([B, NT, NCAND], f32)
            m8l = wp.tile([B, 8], f32)
            si = wp.tile([B, NCAND], f32)
            dmask = wp.tile([B, NT, NCAND], f32)
            dne = wp.tile([B, NT, NCAND], f32)
            dcand = wp.tile([B, NT, NCAND], f32)
            cu = wp.tile([B, NCAND], u32)
            wrow3 = wp.tile([B, NCAND, H + 1], f32)
            prod3 = wp.tile([B, NCAND, H + 1], f32)
            gm8 = wp.tile([B, 8], f32)
            msk = wp.tile([B, NCORES, NCAND], f32)
            gidxf = wp.tile([B, 1], f32)
            gidx = wp.tile([B, 1], u32)
            gx = wp.tile([B, G4], f32)
            gsb = wp.tile([B, G4], f32)
            acts = wp.tile([B, G4], f32)
            fc = wp.tile([B, H], f32)
            ig = wp.tile([B, H], f32)
            tct = wp.tile([B, H], f32)
            h1a = wp.tile([B, H + 1], f32)
            nc.vector.memset(h1a[:, H : H + 1], 1.0)
            h1 = h1a[:, 0:H]

            hT = sp.tile([128, KH, B], f32, tag="hT")
            nc.sync.dma_start(out=hT[:], in_=h0T_p[:].rearrange("(a p) b -> p a b", p=128))
            tc.strict_bb_all_engine_barrier()
            hTr = sp.tile([128, KH, B], f32r, tag="hTr")
            nc.vector.tensor_copy(hTr[:], hT[:])
            c_prev = sp.tile([B, H], f32, tag="c")
            nc.sync.dma_start(out=c_prev[:], in_=c0_p[:])
            tc.strict_bb_all_engine_barrier()

            # ---- gates for t=0: bias + W_hh @ h0 (exact fp32) ----
            gates = pg.tile([B, G4], f32, tag="gates")
            for n in range(4):
                ns = slice(n * 512, (n + 1) * 512)
                nc.tensor.matmul(out=gates[:, ns], lhsT=ones1[:], rhs=bias[:, ns],
                                 start=True, stop=False)
                for k in range(KH):
                    nc.tensor.matmul(out=gates[:, ns], lhsT=hT[:, k, :], rhs=whh[:, k, ns],
                                     start=False, stop=(k == KH - 1))

            first = True
            for t in range(steps):
                # ---- LSTM pointwise; gate layout is [i, f, o, g] ----
                if first:
                    nc.scalar.activation(gsb[:], gates[:], AF.Copy)
                    first = False
                else:
                    nc.vector.tensor_tensor(gsb[:], gates[:], gx[:], op=OP.add)
                nc.scalar.activation(acts[:, 0 : 3 * H], gsb[:, 0 : 3 * H], AF.Sigmoid)
                nc.scalar.activation(acts[:, 3 * H : G4], gsb[:, 3 * H : G4], AF.Tanh)
                nc.vector.tensor_tensor(fc[:], acts[:, H : 2 * H], c_prev[:], op=OP.mult)
                nc.vector.tensor_tensor(ig[:], acts[:, 0:H], acts[:, 3 * H : G4], op=OP.mult)
                c_new = sp.tile([B, H], f32, tag="c")
                nc.vector.tensor_tensor(c_new[:], fc[:], ig[:], op=OP.add)
                nc.scalar.activation(tct[:], c_new[:], AF.Tanh)
                nc.vector.tensor_tensor(h1[:], acts[:, 2 * H : 3 * H], tct[:], op=OP.mult)
                c_prev = c_new

                # ---- h1 -> h1.T tiles (PE transpose); both copies on DVE so
                # the logits (hTr) unblock no later than the gates (hT) ----
                hT = sp.tile([128, KH, B], f32, tag="hT")
                hTr = sp.tile([128, KH, B], f32r, tag="hTr")
                for k in range(KH):
                    tp = pt.tile([128, B], f32, tag="tp")
                    nc.tensor.transpose(out=tp[:], in_=h1[:, k * 128 : (k + 1) * 128],
                                        identity=ident[0:B, 0:B])
                    nc.vector.tensor_copy(hTr[:, k, :], tp[:])
                    nc.vector.tensor_copy(hT[:, k, :], tp[:])

                # ---- logits shard in fp32r + per-tile top-8 scan ----
                logits = kp.tile([B, VS], f32, tag="logits")
                last = t == steps - 1
                for n in range(NT):
                    ns = slice(n * TN, (n + 1) * TN)
                    lg = pl.tile([B, TN], f32, tag="lg")
                    nc.tensor.matmul(out=lg[:], lhsT=ones1r[:], rhs=bout[:, ns],
                                     start=True, stop=False)
                    for k in range(KH):
                        nc.tensor.matmul(out=lg[:], lhsT=hTr[:, k, :], rhs=wout[:, k, ns],
                                         start=False, stop=(k == KH - 1))
                    nc.scalar.activation(logits[:, ns], lg[:], AF.Copy)
                    if not last:
                        m8 = kp.tile([B, 8], f32, tag="m8")
                        i8 = kp.tile([B, 8], u32, tag="i8")
                        nc.vector.max(out=m8[:], in_=logits[:, ns])
                        nc.vector.max_index(out=i8[:], in_max=m8[:], in_values=logits[:, ns])
                        nc.scalar.activation(tv[:, n, :], m8[:, 0:NCAND], AF.Copy)
                        nc.vector.tensor_copy(ti[:, n, :], i8[:, 0:NCAND])
                nc.scalar.dma_start(out=out_p[t], in_=logits[:])
                if last:
                    break

                # ---- gates h-part for t+1 (exact fp32) — PE works under the tail ----
                gates = pg.tile([B, G4], f32, tag="gates")
                for n in range(4):
                    ns = slice(n * 512, (n + 1) * 512)
                    for k in range(KH):
                        nc.tensor.matmul(out=gates[:, ns], lhsT=hT[:, k, :], rhs=whh[:, k, ns],
                                         start=(k == 0), stop=(k == KH - 1))

                # ---- local top-3 (distinct ids) from the 24 pooled candidates ----
                nc.vector.tensor_tensor(ti[:], ti[:], tbase[:].rearrange("b (n c) -> b n c", c=NCAND), op=OP.add)
                nc.vector.max(out=m8l[:], in_=tv[:])
                for k in range(NCAND):
                    nc.vector.tensor_scalar(dmask[:], tv[:], m8l[:, k : k + 1], None, op0=OP.is_ge)
                    for j in range(k):
                        nc.vector.tensor_scalar(dne[:], ti[:], si[:, j : j + 1], None, op0=OP.not_equal)
                        nc.vector.tensor_tensor(dmask[:], dmask[:], dne[:], op=OP.mult)
                    nc.vector.tensor_scalar(dcand[:], dmask[:], -BIG, BIG, op0=OP.mult, op1=OP.add)
                    nc.vector.tensor_tensor(dcand[:], ti[:], dcand[:], op=OP.add)
                    nc.vector.tensor_reduce(si[:, k : k + 1], dcand[:], axis=AX.XY, op=OP.min)
                nc.vector.tensor_copy(pk[:, NCAND : 2 * NCAND], si[:])
                nc.vector.tensor_copy(cu[:], si[:])

                # ---- exact fp32 recheck of the 3 local candidates ----
                for j in range(NCAND):
                    nc.gpsimd.indirect_dma_start(
                        out=wrow3[:, j], out_offset=None, in_=wfullb_p[:],
                        in_offset=bass.IndirectOffsetOnAxis(ap=cu[:, j : j + 1], axis=0),
                    )
                for j in range(NCAND):
                    nc.vector.tensor_tensor(prod3[:, j], wrow3[:, j], h1a[:], op=OP.mult)
                nc.vector.tensor_reduce(pk[:, 0:NCAND], prod3[:], axis=AX.X, op=OP.add)

                # ---- AllGather of (3 exact vals | 3 ids) per core ----
                nc.vector.transpose(pkT[:], pk[:])
                agin = dp.tile([2 * NCAND, B], f32, tag="agin")
                nc.sync.dma_start(out=agin[:, 0:32], in_=pkT[0 : 2 * NCAND, 0:32])
                nc.sync.dma_start(out=agin[:, 32:64], in_=pkT[32 : 32 + 2 * NCAND, 0:32])
                agout = dp.tile([NCORES * 2 * NCAND, B], f32, tag="agout", addr_space="Shared")
                nc.gpsimd.collective_compute(
                    "AllGather", OP.bypass, replica_groups=rg,
                    ins=[agin[:].opt()], outs=[agout[:].opt()],
                )
                # block-swapped readback so one DVE stream-transpose finishes it
                nc.sync.dma_start(out=gpre[0:32, 0:32], in_=agout[0:32, 0:32])
                nc.sync.dma_start(out=gpre[0:16, 32:64], in_=agout[32:48, 0:32])
                nc.sync.dma_start(out=gpre[32:64, 0:32], in_=agout[0:32, 32:64])
                nc.sync.dma_start(out=gpre[32:48, 32:64], in_=agout[32:48, 32:64])
                nc.vector.transpose(gpT[:], gpre[:])
                gall = gpT[:, 0:48].rearrange("b (r s) -> b r s", s=2 * NCAND)
                gv = gall[:, :, 0:NCAND]
                gi = gall[:, :, NCAND : 2 * NCAND]

                # ---- global argmax over 24 exact candidates (min-id tiebreak) ----
                nc.vector.max(out=gm8[:], in_=gv)
                nc.vector.tensor_scalar(msk[:], gv, gm8[:, 0:1], None, op0=OP.is_ge)
                nc.vector.tensor_scalar(msk[:], msk[:], -BIG, BIG, op0=OP.mult, op1=OP.add)
                nc.vector.tensor_tensor(msk[:], gi, msk[:], op=OP.add)
                nc.vector.tensor_reduce(gidxf[:], msk[:], axis=AX.XY, op=OP.min)
                nc.vector.tensor_copy(gidx[:], gidxf[:])

                # ---- gather the token's precomputed gate row ----
                nc.gpsimd.indirect_dma_start(
                    out=gx[:], out_offset=None, in_=whe_p[:],
                    in_offset=bass.IndirectOffsetOnAxis(ap=gidx[:, :1], axis=0),
                )

    nc.finalize()  # Bacc: runs compile() legalization passes
    return nc


# gate-unit permutation [i, f, o, g] (torch order in the weights is i, f, g, o)
_PERM = np.concatenate([np.arange(0, 1024), np.arange(1536, 2048), np.arange(1024, 1536)])


def make_in_maps(inputs):
    inp = {k: np.asarray(v) for k, v in inputs.items()}
    h0 = inp["h0"].astype(np.float32)
    c0 = inp["c0"].astype(np.float32)
    W_ih = inp["W_ih"].astype(np.float32)
    W_hh = inp["W_hh"].astype(np.float32)
    b = (inp["b_ih"].astype(np.float32) + inp["b_hh"].astype(np.float32))
    W_out = inp["W_out"].astype(np.float32)
    b_out = inp["b_out"].astype(np.float32)
    emb = inp["embed_table"].astype(np.float32)
    # x @ W_ih.T + b for every vocab row, fp32, gate units permuted to [i,f,o,g]
    whe = np.ascontiguousarray((emb @ W_ih.T + b)[:, _PERM].astype(np.float32))
    wfullb = np.ascontiguousarray(
        np.concatenate([W_out, b_out.reshape(V, 1)], axis=1).astype(np.float32))
    whhT = np.ascontiguousarray(W_hh[_PERM].T)
    bias = np.ascontiguousarray(b[_PERM].reshape(1, G4))
    in_maps = []
    for c in range(NCORES):
        base = c * VS
        tbase = np.zeros((B, NT * NCAND), np.float32)
        for n in range(NT):
            tbase[:, NCAND * n : NCAND * (n + 1)] = float(base + n * TN)
        in_maps.append({
            "h0T": np.ascontiguousarray(h0.T),
            "c0": np.ascontiguousarray(c0),
            "whhT": whhT,
            "bias": bias,
            "woutT": np.ascontiguousarray(W_out[base : base + VS].T),
            "bout": np.ascontiguousarray(b_out[base : base + VS].reshape(1, VS)),
            "whe": whe,
            "wfullb": wfullb,
            "tbase": tbase,
        })
    return in_maps


def run(inputs, steps=S, trace=False):
    from concourse.bass_utils import run_bass_kernel_spmd

    nc = build_program(steps)
    res = run_bass_kernel_spmd(nc, make_in_maps(inputs), list(range(NCORES)),
                               trace=trace)
    outs = [res.results[c]["out"] for c in range(NCORES)]      # each [steps, B, VS]
    full = np.concatenate(outs, axis=2)                        # [steps, B, V]
    return np.ascontiguousarray(np.transpose(full, (1, 0, 2))), res


def kernel(**inputs):
    out, _ = run(inputs, steps=S, trace=False)
    return out.astype(np.float32)


# revision 14
# speedup vs baseline: 1.3189x; 1.1065x over previous
"""LSTM greedy decoder on 8 trn2 NeuronCores.

Vocab-parallel: each core keeps a resident fp32r SBUF copy of its
4000-row W_out shard, replicates the LSTM cell (exact fp32), and agrees
on the greedy token by exchanging per-core top-3 candidates via a tiny
AllGather.

Scheduling notes:
- per-tile MAX8/MAX_INDEX8 (true sorted top-8) run pipelined under the
  logits matmuls; a short mask dance then picks the core's top-3
  distinct candidates from the pooled per-tile top-3s.
- the top-3 candidate logits are re-evaluated exactly in fp32 BEFORE
  the AllGather (indirect row gathers + fp32 dots), so the exchanged
  values are exact and the global step is a plain max.
- AllGather staging uses DVE 32x32 stream-transposes + contiguous DMAs
  (a naive layout generates thousands of 4-byte descriptors).
- gates matmuls for step t+1 are emitted before the exchange tail so
  the PE works under the collective; no PE instructions appear in the
  tail (PE queues are FIFO — a late transpose would block those gates).
- all single-buffer tail scratch tiles are allocated once, outside the
  step loop: per-iteration bufs=1 re-allocation hits the tile
  framework's min-join fallback, which stalls the allocation until
  every engine (including the ~30us-behind Tensor queue) passes the
  previous release.
- gate layout is host-permuted to [i, f, o, g] so one fused Sigmoid
  covers i/f/o; the x-contribution table whe = embed @ W_ih.T + b is
  gathered per token and added in a single [B, 4H] op.
"""

import numpy as np

B, H, D, V, S = 64, 512, 256, 32000, 64
NCORES = 8
VS = V // NCORES            # 4000 vocab rows per core
G4 = 4 * H                  # 2048 gate units
NT = 8                      # logits N-tiles per step
TN = VS // NT               # 500 columns per logits tile
KH = H // 128               # 4 contraction tiles over H
BIG = 1.0e9
NCAND = 2                   # exact-rechecked candidates per core
TPT = 3                     # per-tile candidates pooled


def build_program(steps=S):
    import concourse.bass as bass
    import concourse.bacc as bacc
    import concourse.mybir as mybir
    import concourse.tile as tile
    from concourse.masks import make_identity

    f32 = mybir.dt.float32
    f32r = mybir.dt.float32r
    u32 = mybir.dt.uint32
    AF = mybir.ActivationFunctionType
    OP = mybir.AluOpType
    AX = mybir.AxisListType

    nc = bacc.Bacc(num_devices=NCORES)
    h0T_p = nc.declare_dram_parameter("h0T", [H, B], f32, isOutput=False)
    c0_p = nc.declare_dram_parameter("c0", [B, H], f32, isOutput=False)
    whhT_p = nc.declare_dram_parameter("whhT", [H, G4], f32, isOutput=False)
    bias_p = nc.declare_dram_parameter("bias", [1, G4], f32, isOutput=False)
    woutT_p = nc.declare_dram_parameter("woutT", [H, VS], f32, isOutput=False)
    bout_p = nc.declare_dram_parameter("bout", [1, VS], f32, isOutput=False)
    whe_p = nc.declare_dram_parameter("whe", [V, G4], f32, isOutput=False)
    wfullb_p = nc.declare_dram_parameter("wfullb", [V, H + 1], f32, isOutput=False)
    tbase_p = nc.declare_dram_parameter("tbase", [B, NT * TPT], f32, isOutput=False)
    out_p = nc.declare_dram_parameter("out", [steps, B, VS], f32, isOutput=True)

    rg = [list(range(NCORES))]

    with tile.TileContext(nc) as tc:
        with (
            tc.tile_pool(name="wpool", bufs=1) as wp,
            tc.tile_pool(name="state", bufs=2) as sp,
            tc.tile_pool(name="work", bufs=2) as kp,
            tc.tile_pool(name="ps_g", bufs=1, space="PSUM") as pg,
            tc.tile_pool(name="ps_l", bufs=2, space="PSUM") as pl,
            tc.tile_pool(name="ps_t", bufs=2, space="PSUM") as pt,
            tc.tile_pool(name="dram", bufs=2, space="DRAM") as dp,
        ):
            # ---- constants (engine-local, no DMA) ----
            ident = wp.tile([128, 128], f32)
            make_identity(nc, ident[:])
            ones1 = wp.tile([1, B], f32)
            nc.vector.memset(ones1[:], 1.0)
            ones1r = wp.tile([1, B], f32r)
            nc.vector.tensor_copy(ones1r[:], ones1[:])

            # ---- resident weights (barriers cap per-inst sync-wait fan-in) ----
            tc.strict_bb_all_engine_barrier()
            whh = wp.tile([128, KH, G4], f32)
            nc.sync.dma_start(out=whh[:], in_=whhT_p[:].rearrange("(a p) n -> p a n", p=128))
            bias = wp.tile([1, G4], f32)
            nc.sync.dma_start(out=bias[:], in_=bias_p[:])
            tbase = wp.tile([B, NT * TPT], f32)
            nc.sync.dma_start(out=tbase[:], in_=tbase_p[:])
            tc.strict_bb_all_engine_barrier()

            # fp32r weights: stage fp32 chunks through the logits-tagged slot,
            # then round-copy (walrus requires fp32r-matmul inputs pre-rounded)
            wout = wp.tile([128, KH, VS], f32r)
            bout = wp.tile([1, VS], f32r)
            wq = woutT_p[:].rearrange("(a p) n -> p a n", p=128)
            for k in range(KH):
                stage = kp.tile([128, VS], f32, tag="logits", name=f"wstage{k}")
                nc.sync.dma_start(out=stage[:], in_=wq[:, k, :])
                nc.vector.tensor_copy(wout[:, k, :], stage[:])
                if k == 0:
                    bstage = kp.tile([1, VS], f32, tag="logits", name="bstage")
                    nc.sync.dma_start(out=bstage[:], in_=bout_p[:])
                    nc.vector.tensor_copy(bout[:], bstage[:])
                tc.strict_bb_all_engine_barrier()

            # ---- loop-invariant scratch (alloc once: per-iter bufs=1
            # re-allocs hit the min-join fallback and stall on Tensor) ----
            pk = wp.tile([B, 32], f32)          # [ev0..2 | id0..2 | pad]
            nc.vector.memset(pk[:], 0.0)
            pkT = wp.tile([B, 32], f32)
            gpre = wp.tile([B, 32], f32)
            nc.vector.memset(gpre[:], 0.0)
            gpT = wp.tile([B, 32], f32)
            tv = wp.tile([B, NT, TPT], f32)
            ti = wp.tile([B, NT, TPT], f32)
            m8l = wp.tile([B, 8], f32)
            si = wp.tile([B, NCAND], f32)
            dmask = wp.tile([B, NT, TPT], f32)
            dne = wp.tile([B, NT, TPT], f32)
            dcand = wp.tile([B, NT, TPT], f32)
            cu = wp.tile([B, NCAND], u32)
            wrow3 = wp.tile([B, NCAND, H + 1], f32)
            prod3 = wp.tile([B, NCAND, H + 1], f32)
            gm8 = wp.tile([B, 8], f32)
            msk = wp.tile([B, NCORES, NCAND], f32)
            gidxf = wp.tile([B, 1], f32)
            gidx = wp.tile([B, 1], u32)
            gx = wp.tile([B, G4], f32)
            gsb = wp.tile([B, G4], f32)
            acts = wp.tile([B, G4], f32)
            fc = wp.tile([B, H], f32)
            ig = wp.tile([B, H], f32)
            tct = wp.tile([B, H], f32)
            h1a = wp.tile([B, H + 1], f32)
            nc.vector.memset(h1a[:, H : H + 1], 1.0)
            h1 = h1a[:, 0:H]

            hT = sp.tile([128, KH, B], f32, tag="hT")
            nc.sync.dma_start(out=hT[:], in_=h0T_p[:].rearrange("(a p) b -> p a b", p=128))
            tc.strict_bb_all_engine_barrier()
            hTr = sp.tile([128, KH, B], f32r, tag="hTr")
            nc.vector.tensor_copy(hTr[:], hT[:])
            c_prev = sp.tile([B, H], f32, tag="c")
            nc.sync.dma_start(out=c_prev[:], in_=c0_p[:])
            tc.strict_bb_all_engine_barrier()

            # ---- gates for t=0: bias + W_hh @ h0 (exact fp32) ----
            gates = pg.tile([B, G4], f32, tag="gates")
            for n in range(4):
                ns = slice(n * 512, (n + 1) * 512)
                nc.tensor.matmul(out=gates[:, ns], lhsT=ones1[:], rhs=bias[:, ns],
                                 start=True, stop=False)
                for k in range(KH):
                    nc.tensor.matmul(out=gates[:, ns], lhsT=hT[:, k, :], rhs=whh[:, k, ns],
                                     start=False, stop=(k == KH - 1))

            first = True
            for t in range(steps):
                # ---- LSTM pointwise; gate layout is [i, f, o, g] ----
                if first:
                    nc.scalar.activation(gsb[:], gates[:], AF.Copy)
                    first = False
                else:
                    nc.vector.tensor_tensor(gsb[:, 0 : 3 * H], gates[:, 0 : 3 * H], gx[:, 0 : 3 * H], op=OP.add)
                    nc.vector.tensor_tensor(gsb[:, 3 * H : G4], gates[:, 3 * H : G4], gx[:, 3 * H : G4], op=OP.add)
                nc.scalar.activation(acts[:, 0 : 3 * H], gsb[:, 0 : 3 * H], AF.Sigmoid)
                nc.scalar.activation(acts[:, 3 * H : G4], gsb[:, 3 * H : G4], AF.Tanh)
                nc.gpsimd.tensor_tensor(fc[:], acts[:, H : 2 * H], c_prev[:], op=OP.mult)
                nc.vector.tensor_tensor(ig[:], acts[:, 0:H], acts[:, 3 * H : G4], op=OP.mult)
                c_new = sp.tile([B, H], f32, tag="c")
                nc.vector.tensor_tensor(c_new[:], fc[:], ig[:], op=OP.add)
                nc.scalar.activation(tct[:], c_new[:], AF.Tanh)
                nc.vector.tensor_tensor(h1[:], acts[:, 2 * H : 3 * H], tct[:], op=OP.mult)
                c_prev = c_new

                # ---- h1 -> h1.T tiles (PE transpose); both copies on DVE so
                # the logits (hTr) unblock no later than the gates (hT) ----
                hT = sp.tile([128, KH, B], f32, tag="hT")
                hTr = sp.tile([128, KH, B], f32r, tag="hTr")
                for k in range(KH):
                    tp = pt.tile([128, B], f32, tag="tp")
                    nc.tensor.transpose(out=tp[:], in_=h1[:, k * 128 : (k + 1) * 128],
                                        identity=ident[0:B, 0:B])
                    nc.vector.tensor_copy(hTr[:, k, :], tp[:])
                    nc.vector.tensor_copy(hT[:, k, :], tp[:])

                # ---- logits shard in fp32r + per-tile top-8 scan ----
                logits = kp.tile([B, VS], f32, tag="logits")
                last = t == steps - 1
                for n in range(NT):
                    ns = slice(n * TN, (n + 1) * TN)
                    lg = pl.tile([B, TN], f32, tag="lg")
                    nc.tensor.matmul(out=lg[:], lhsT=ones1r[:], rhs=bout[:, ns],
                                     start=True, stop=False)
                    for k in range(KH):
                        nc.tensor.matmul(out=lg[:], lhsT=hTr[:, k, :], rhs=wout[:, k, ns],
                                         start=False, stop=(k == KH - 1))
                    nc.scalar.activation(logits[:, ns], lg[:], AF.Copy)
                    if not last:
                        m8 = kp.tile([B, 8], f32, tag="m8")
                        i8 = kp.tile([B, 8], u32, tag="i8")
                        nc.vector.max(out=m8[:], in_=logits[:, ns])
                        nc.vector.max_index(out=i8[:], in_max=m8[:], in_values=logits[:, ns])
                        nc.scalar.activation(tv[:, n, :], m8[:, 0:TPT], AF.Copy)
                        nc.vector.tensor_copy(ti[:, n, :], i8[:, 0:TPT])
                nc.scalar.dma_start(out=out_p[t], in_=logits[:])
                if last:
                    break

                # ---- gates h-part for t+1 (exact fp32) — PE works under the tail ----
                gates = pg.tile([B, G4], f32, tag="gates")
                for n in range(4):
                    ns = slice(n * 512, (n + 1) * 512)
                    for k in range(KH):
                        nc.tensor.matmul(out=gates[:, ns], lhsT=hT[:, k, :], rhs=whh[:, k, ns],
                                         start=(k == 0), stop=(k == KH - 1))

                # ---- local top-3 (distinct ids) from the 24 pooled candidates ----
                nc.vector.tensor_tensor(ti[:], ti[:], tbase[:].rearrange("b (n c) -> b n c", c=TPT), op=OP.add)
                nc.vector.max(out=m8l[:], in_=tv[:])
                for k in range(NCAND):
                    nc.vector.tensor_scalar(dmask[:], tv[:], m8l[:, k : k + 1], None, op0=OP.is_ge)
                    for j in range(k):
                        nc.vector.tensor_scalar(dne[:], ti[:], si[:, j : j + 1], None, op0=OP.not_equal)
                        nc.vector.tensor_tensor(dmask[:], dmask[:], dne[:], op=OP.mult)
                    nc.vector.tensor_scalar(dcand[:], dmask[:], -BIG, BIG, op0=OP.mult, op1=OP.add)
                    nc.vector.tensor_tensor(dcand[:], ti[:], dcand[:], op=OP.add)
                    nc.vector.tensor_reduce(si[:, k : k + 1], dcand[:], axis=AX.XY, op=OP.min)
                nc.vector.tensor_copy(pk[:, NCAND : 2 * NCAND], si[:])
                nc.vector.tensor_copy(cu[:], si[:])

                # ---- exact fp32 recheck of the 3 local candidates ----
                for j in range(NCAND):
                    nc.gpsimd.indirect_dma_start(
                        out=wrow3[:, j], out_offset=None, in_=wfullb_p[:],
                        in_offset=bass.IndirectOffsetOnAxis(ap=cu[:, j : j + 1], axis=0),
                    )
                for j in range(NCAND):
                    nc.vector.tensor_tensor(prod3[:, j], wrow3[:, j], h1a[:], op=OP.mult)
                nc.vector.tensor_reduce(pk[:, 0:NCAND], prod3[:], axis=AX.X, op=OP.add)

                # PE warmers: dummy transposes dep'd on tail tiles shorten the
                # HAM-throttled idle windows so the next logits phase runs warm
                wm1 = pt.tile([128, B], f32, tag="tp")
                nc.tensor.transpose(out=wm1[0:32, :], in_=pk[:], identity=ident[0:B, 0:B])

                # ---- AllGather of (3 exact vals | 3 ids) per core ----
                nc.vector.transpose(pkT[:], pk[:])
                agin = dp.tile([2 * NCAND, B], f32, tag="agin")
                nc.sync.dma_start(out=agin[:, 0:32], in_=pkT[0 : 2 * NCAND, 0:32])
                nc.scalar.dma_start(out=agin[:, 32:64], in_=pkT[32 : 32 + 2 * NCAND, 0:32])
                agout = dp.tile([NCORES * 2 * NCAND, B], f32, tag="agout", addr_space="Shared")
                nc.gpsimd.collective_compute(
                    "AllGather", OP.bypass, replica_groups=rg,
                    ins=[agin[:].opt()], outs=[agout[:].opt()],
                )
                # block-swapped readback so one DVE stream-transpose finishes it
                nc.sync.dma_start(out=gpre[0:32, 0:32], in_=agout[0:32, 0:32])
                nc.scalar.dma_start(out=gpre[32:64, 0:32], in_=agout[0:32, 32:64])
                wm2 = pt.tile([128, B], f32, tag="tp")
                nc.tensor.transpose(out=wm2[0:32, :], in_=gpre[:], identity=ident[0:B, 0:B])
                nc.vector.transpose(gpT[:], gpre[:])
                gall = gpT[:, 0:32].rearrange("b (r s) -> b r s", s=2 * NCAND)
                gv = gall[:, :, 0:NCAND]
                gi = gall[:, :, NCAND : 2 * NCAND]

                # ---- global argmax over 24 exact candidates (min-id tiebreak) ----
                nc.vector.max(out=gm8[:], in_=gv)
                nc.vector.tensor_scalar(msk[:], gv, gm8[:, 0:1], None, op0=OP.is_ge)
                nc.vector.tensor_scalar(msk[:], msk[:], -BIG, BIG, op0=OP.mult, op1=OP.add)
                nc.vector.tensor_tensor(msk[:], gi, msk[:], op=OP.add)
                nc.vector.tensor_reduce(gidxf[:], msk[:], axis=AX.XY, op=OP.min)
                nc.vector.tensor_copy(gidx[:], gidxf[:])

                # ---- gather the token's precomputed gate row ----
                nc.gpsimd.indirect_dma_start(
                    out=gx[:], out_offset=None, in_=whe_p[:],
                    in_offset=bass.IndirectOffsetOnAxis(ap=gidx[:, :1], axis=0),
                )
                wm3 = pt.tile([128, B], f32, tag="tp")
                nc.tensor.transpose(out=wm3[:], in_=gx[:, 0:128], identity=ident[0:B, 0:B])
                wm4 = pt.tile([128, B], f32, tag="tp")
                nc.tensor.transpose(out=wm4[:], in_=gx[:, 128:256], identity=ident[0:B, 0:B])

    nc.finalize()  # Bacc: runs compile() legalization passes
    return nc


# gate-unit permutation [i, f, o, g] (torch order in the weights is i, f, g, o)
_PERM = np.concatenate([np.arange(0, 1024), np.arange(1536, 2048), np.arange(1024, 1536)])


def make_in_maps(inputs):
    inp = {k: np.asarray(v) for k, v in inputs.items()}
    h0 = inp["h0"].astype(np.float32)
    c0 = inp["c0"].astype(np.float32)
    W_ih = inp["W_ih"].astype(np.float32)
    W_hh = inp["W_hh"].astype(np.float32)
    b = (inp["b_ih"].astype(np.float32) + inp["b_hh"].astype(np.float32))
    W_out = inp["W_out"].astype(np.float32)
    b_out = inp["b_out"].astype(np.float32)
    emb = inp["embed_table"].astype(np.float32)
    # x @ W_ih.T + b for every vocab row, fp32, gate units permuted to [i,f,o,g]
    whe = np.ascontiguousarray((emb @ W_ih.T + b)[:, _PERM].astype(np.float32))
    wfullb = np.ascontiguousarray(
        np.concatenate([W_out, b_out.reshape(V, 1)], axis=1).astype(np.float32))
    whhT = np.ascontiguousarray(W_hh[_PERM].T)
    bias = np.ascontiguousarray(b[_PERM].reshape(1, G4))
    in_maps = []
    for c in range(NCORES):
        base = c * VS
        tbase = np.zeros((B, NT * TPT), np.float32)
        for n in range(NT):
            tbase[:, TPT * n : TPT * (n + 1)] = float(base + n * TN)
        in_maps.append({
            "h0T": np.ascontiguousarray(h0.T),
            "c0": np.ascontiguousarray(c0),
            "whhT": whhT,
            "bias": bias,
            "woutT": np.ascontiguousarray(W_out[base : base + VS].T),
            "bout": np.ascontiguousarray(b_out[base : base + VS].reshape(1, VS)),
            "whe": whe,
            "wfullb": wfullb,
            "tbase": tbase,
        })
    return in_maps


def run(inputs, steps=S, trace=False):
    from concourse.bass_utils import run_bass_kernel_spmd

    nc = build_program(steps)
    res = run_bass_kernel_spmd(nc, make_in_maps(inputs), list(range(NCORES)),
                               trace=trace)
    outs = [res.results[c]["out"] for c in range(NCORES)]      # each [steps, B, VS]
    full = np.concatenate(outs, axis=2)                        # [steps, B, V]
    return np.ascontiguousarray(np.transpose(full, (1, 0, 2))), res


def kernel(**inputs):
    out, _ = run(inputs, steps=S, trace=False)
    return out.astype(np.float32)


# revision 15
# speedup vs baseline: 1.3253x; 1.0048x over previous
"""LSTM greedy decoder on 8 trn2 NeuronCores.

Vocab-parallel: each core keeps a resident fp32r SBUF copy of its
4000-row W_out shard, replicates the LSTM cell (exact fp32), and agrees
on the greedy token by exchanging per-core top-3 candidates via a tiny
AllGather.

Scheduling notes:
- per-tile MAX8/MAX_INDEX8 (true sorted top-8) run pipelined under the
  logits matmuls; a short mask dance then picks the core's top-3
  distinct candidates from the pooled per-tile top-3s.
- the top-3 candidate logits are re-evaluated exactly in fp32 BEFORE
  the AllGather (indirect row gathers + fp32 dots), so the exchanged
  values are exact and the global step is a plain max.
- AllGather staging uses DVE 32x32 stream-transposes + contiguous DMAs
  (a naive layout generates thousands of 4-byte descriptors).
- gates matmuls for step t+1 are emitted before the exchange tail so
  the PE works under the collective; no PE instructions appear in the
  tail (PE queues are FIFO — a late transpose would block those gates).
- all single-buffer tail scratch tiles are allocated once, outside the
  step loop: per-iteration bufs=1 re-allocation hits the tile
  framework's min-join fallback, which stalls the allocation until
  every engine (including the ~30us-behind Tensor queue) passes the
  previous release.
- gate layout is host-permuted to [i, f, o, g] so one fused Sigmoid
  covers i/f/o; the x-contribution table whe = embed @ W_ih.T + b is
  gathered per token and added in a single [B, 4H] op.
"""

import numpy as np

B, H, D, V, S = 64, 512, 256, 32000, 64
NCORES = 8
VS = V // NCORES            # 4000 vocab rows per core
G4 = 4 * H                  # 2048 gate units
NT = 8                      # logits N-tiles per step
TN = VS // NT               # 500 columns per logits tile
KH = H // 128               # 4 contraction tiles over H
BIG = 1.0e9
NCAND = 2                   # exact-rechecked candidates per core
TPT = 3                     # per-tile candidates pooled


def build_program(steps=S):
    import concourse.bass as bass
    import concourse.bacc as bacc
    import concourse.mybir as mybir
    import concourse.tile as tile
    from concourse.masks import make_identity

    f32 = mybir.dt.float32
    f32r = mybir.dt.float32r
    u32 = mybir.dt.uint32
    AF = mybir.ActivationFunctionType
    OP = mybir.AluOpType
    AX = mybir.AxisListType

    nc = bacc.Bacc(num_devices=NCORES)
    h0T_p = nc.declare_dram_parameter("h0T", [H, B], f32, isOutput=False)
    c0_p = nc.declare_dram_parameter("c0", [B, H], f32, isOutput=False)
    whhT_p = nc.declare_dram_parameter("whhT", [H, G4], f32, isOutput=False)
    bias_p = nc.declare_dram_parameter("bias", [1, G4], f32, isOutput=False)
    woutT_p = nc.declare_dram_parameter("woutT", [H, VS], f32, isOutput=False)
    bout_p = nc.declare_dram_parameter("bout", [1, VS], f32, isOutput=False)
    whe_p = nc.declare_dram_parameter("whe", [V, G4], f32, isOutput=False)
    wfullb_p = nc.declare_dram_parameter("wfullb", [V, H + 1], f32, isOutput=False)
    tbase_p = nc.declare_dram_parameter("tbase", [B, NT * TPT], f32, isOutput=False)
    out_p = nc.declare_dram_parameter("out", [steps, B, VS], f32, isOutput=True)

    rg = [list(range(NCORES))]

    with tile.TileContext(nc) as tc:
        with (
            tc.tile_pool(name="wpool", bufs=1) as wp,
            tc.tile_pool(name="state", bufs=2) as sp,
            tc.tile_pool(name="work", bufs=2) as kp,
            tc.tile_pool(name="ps_g", bufs=1, space="PSUM") as pg,
            tc.tile_pool(name="ps_l", bufs=2, space="PSUM") as pl,
            tc.tile_pool(name="ps_t", bufs=2, space="PSUM") as pt,
            tc.tile_pool(name="dram", bufs=2, space="DRAM") as dp,
        ):
            # ---- constants (engine-local, no DMA) ----
            ident = wp.tile([128, 128], f32)
            make_identity(nc, ident[:])
            ones1 = wp.tile([1, B], f32)
            nc.vector.memset(ones1[:], 1.0)
            ones1r = wp.tile([1, B], f32r)
            nc.vector.tensor_copy(ones1r[:], ones1[:])

            # ---- resident weights (barriers cap per-inst sync-wait fan-in) ----
            tc.strict_bb_all_engine_barrier()
            whh = wp.tile([128, KH, G4], f32)
            nc.sync.dma_start(out=whh[:], in_=whhT_p[:].rearrange("(a p) n -> p a n", p=128))
            bias = wp.tile([1, G4], f32)
            nc.sync.dma_start(out=bias[:], in_=bias_p[:])
            tbase = wp.tile([B, NT * TPT], f32)
            nc.sync.dma_start(out=tbase[:], in_=tbase_p[:])
            tc.strict_bb_all_engine_barrier()

            # fp32r weights: stage fp32 chunks through the logits-tagged slot,
            # then round-copy (walrus requires fp32r-matmul inputs pre-rounded)
            wout = wp.tile([128, KH, VS], f32r)
            bout = wp.tile([1, VS], f32r)
            wq = woutT_p[:].rearrange("(a p) n -> p a n", p=128)
            for k in range(KH):
                stage = kp.tile([128, VS], f32, tag="logits", name=f"wstage{k}")
                nc.sync.dma_start(out=stage[:], in_=wq[:, k, :])
                nc.vector.tensor_copy(wout[:, k, :], stage[:])
                if k == 0:
                    bstage = kp.tile([1, VS], f32, tag="logits", name="bstage")
                    nc.sync.dma_start(out=bstage[:], in_=bout_p[:])
                    nc.vector.tensor_copy(bout[:], bstage[:])
                tc.strict_bb_all_engine_barrier()

            # ---- loop-invariant scratch (alloc once: per-iter bufs=1
            # re-allocs hit the min-join fallback and stall on Tensor) ----
            pk = wp.tile([B, 32], f32)          # [ev0..2 | id0..2 | pad]
            nc.vector.memset(pk[:], 0.0)
            pkT = wp.tile([B, 32], f32)
            gpre = wp.tile([B, 32], f32)
            nc.vector.memset(gpre[:], 0.0)
            gpT = wp.tile([B, 32], f32)
            tv = wp.tile([B, NT, TPT], f32)
            ti = wp.tile([B, NT, TPT], f32)
            m8l = wp.tile([B, 8], f32)
            si = wp.tile([B, NCAND], f32)
            dmask = wp.tile([B, NT, TPT], f32)
            dne = wp.tile([B, NT, TPT], f32)
            dcand = wp.tile([B, NT, TPT], f32)
            cu = wp.tile([B, NCAND], u32)
            wrow3 = wp.tile([B, NCAND, H + 1], f32)
            prod3 = wp.tile([B, NCAND, H + 1], f32)
            gm8 = wp.tile([B, 8], f32)
            msk = wp.tile([B, NCORES, NCAND], f32)
            gidxf = wp.tile([B, 1], f32)
            gidx = wp.tile([B, 1], u32)
            gx = wp.tile([B, G4], f32)
            gsb = wp.tile([B, G4], f32)
            acts = wp.tile([B, G4], f32)
            fc = wp.tile([B, H], f32)
            ig = wp.tile([B, H], f32)
            tct = wp.tile([B, H], f32)
            h1a = wp.tile([B, H + 1], f32)
            nc.vector.memset(h1a[:, H : H + 1], 1.0)
            h1 = h1a[:, 0:H]

            hT = sp.tile([128, KH, B], f32, tag="hT")
            nc.sync.dma_start(out=hT[:], in_=h0T_p[:].rearrange("(a p) b -> p a b", p=128))
            tc.strict_bb_all_engine_barrier()
            hTr = sp.tile([128, KH, B], f32r, tag="hTr")
            nc.vector.tensor_copy(hTr[:], hT[:])
            c_prev = sp.tile([B, H], f32, tag="c")
            nc.sync.dma_start(out=c_prev[:], in_=c0_p[:])
            tc.strict_bb_all_engine_barrier()

            # ---- gates for t=0: bias + W_hh @ h0 (exact fp32) ----
            gates = pg.tile([B, G4], f32, tag="gates")
            for n in range(4):
                ns = slice(n * 512, (n + 1) * 512)
                nc.tensor.matmul(out=gates[:, ns], lhsT=ones1[:], rhs=bias[:, ns],
                                 start=True, stop=False)
                for k in range(KH):
                    nc.tensor.matmul(out=gates[:, ns], lhsT=hT[:, k, :], rhs=whh[:, k, ns],
                                     start=False, stop=(k == KH - 1))

            first = True
            for t in range(steps):
                # ---- LSTM pointwise; gate layout is [i, f, o, g] ----
                if first:
                    nc.scalar.activation(gsb[:], gates[:], AF.Copy)
                    first = False
                else:
                    nc.vector.tensor_tensor(gsb[:, 0 : 3 * H], gates[:, 0 : 3 * H], gx[:, 0 : 3 * H], op=OP.add)
                    nc.vector.tensor_tensor(gsb[:, 3 * H : G4], gates[:, 3 * H : G4], gx[:, 3 * H : G4], op=OP.add)
                nc.scalar.activation(acts[:, 0 : 3 * H], gsb[:, 0 : 3 * H], AF.Sigmoid)
                nc.scalar.activation(acts[:, 3 * H : G4], gsb[:, 3 * H : G4], AF.Tanh)
                # dense PE warm burst under the pointwise window: ~3.5us of
                # back-to-back dummy transposes un-throttle the HAM clock gate
                # so the logits matmuls run at 2.4GHz instead of 1.2GHz
                for w in range(9):
                    wmt = pt.tile([128, B], f32, tag="tp")
                    nc.tensor.transpose(out=wmt[:], in_=acts[:, 128 * w : 128 * (w + 1)],
                                        identity=ident[0:B, 0:B])
                nc.gpsimd.tensor_tensor(fc[:], acts[:, H : 2 * H], c_prev[:], op=OP.mult)
                nc.vector.tensor_tensor(ig[:], acts[:, 0:H], acts[:, 3 * H : G4], op=OP.mult)
                c_new = sp.tile([B, H], f32, tag="c")
                nc.vector.tensor_tensor(c_new[:], fc[:], ig[:], op=OP.add)
                nc.scalar.activation(tct[:], c_new[:], AF.Tanh)
                nc.vector.tensor_tensor(h1[:], acts[:, 2 * H : 3 * H], tct[:], op=OP.mult)
                c_prev = c_new

                # ---- h1 -> h1.T tiles (PE transpose); both copies on DVE so
                # the logits (hTr) unblock no later than the gates (hT) ----
                hT = sp.tile([128, KH, B], f32, tag="hT")
                hTr = sp.tile([128, KH, B], f32r, tag="hTr")
                for k in range(KH):
                    tp = pt.tile([128, B], f32, tag="tp")
                    nc.tensor.transpose(out=tp[:], in_=h1[:, k * 128 : (k + 1) * 128],
                                        identity=ident[0:B, 0:B])
                    nc.vector.tensor_copy(hTr[:, k, :], tp[:])
                    nc.vector.tensor_copy(hT[:, k, :], tp[:])

                # ---- logits shard in fp32r + per-tile top-8 scan ----
                logits = kp.tile([B, VS], f32, tag="logits")
                last = t == steps - 1
                for n in range(NT):
                    ns = slice(n * TN, (n + 1) * TN)
                    lg = pl.tile([B, TN], f32, tag="lg")
                    nc.tensor.matmul(out=lg[:], lhsT=ones1r[:], rhs=bout[:, ns],
                                     start=True, stop=False)
                    for k in range(KH):
                        nc.tensor.matmul(out=lg[:], lhsT=hTr[:, k, :], rhs=wout[:, k, ns],
                                         start=False, stop=(k == KH - 1))
                    nc.scalar.activation(logits[:, ns], lg[:], AF.Copy)
                    if not last:
                        m8 = kp.tile([B, 8], f32, tag="m8")
                        i8 = kp.tile([B, 8], u32, tag="i8")
                        nc.vector.max(out=m8[:], in_=logits[:, ns])
                        nc.vector.max_index(out=i8[:], in_max=m8[:], in_values=logits[:, ns])
                        nc.scalar.activation(tv[:, n, :], m8[:, 0:TPT], AF.Copy)
                        nc.vector.tensor_copy(ti[:, n, :], i8[:, 0:TPT])
                nc.scalar.dma_start(out=out_p[t], in_=logits[:])
                if last:
                    break

                # ---- gates h-part for t+1 (exact fp32) — PE works under the tail ----
                gates = pg.tile([B, G4], f32, tag="gates")
                for n in range(4):
                    ns = slice(n * 512, (n + 1) * 512)
                    for k in range(KH):
                        nc.tensor.matmul(out=gates[:, ns], lhsT=hT[:, k, :], rhs=whh[:, k, ns],
                                         start=(k == 0), stop=(k == KH - 1))

                # ---- local top-3 (distinct ids) from the 24 pooled candidates ----
                nc.vector.tensor_tensor(ti[:], ti[:], tbase[:].rearrange("b (n c) -> b n c", c=TPT), op=OP.add)
                nc.vector.max(out=m8l[:], in_=tv[:])
                for k in range(NCAND):
                    nc.vector.tensor_scalar(dmask[:], tv[:], m8l[:, k : k + 1], None, op0=OP.is_ge)
                    for j in range(k):
                        nc.vector.tensor_scalar(dne[:], ti[:], si[:, j : j + 1], None, op0=OP.not_equal)
                        nc.vector.tensor_tensor(dmask[:], dmask[:], dne[:], op=OP.mult)
                    nc.vector.tensor_scalar(dcand[:], dmask[:], -BIG, BIG, op0=OP.mult, op1=OP.add)
                    nc.vector.tensor_tensor(dcand[:], ti[:], dcand[:], op=OP.add)
                    nc.vector.tensor_reduce(si[:, k : k + 1], dcand[:], axis=AX.XY, op=OP.min)
                nc.vector.tensor_copy(pk[:, NCAND : 2 * NCAND], si[:])
                nc.vector.tensor_copy(cu[:], si[:])

                # ---- exact fp32 recheck of the 3 local candidates ----
                for j in range(NCAND):
                    nc.gpsimd.indirect_dma_start(
                        out=wrow3[:, j], out_offset=None, in_=wfullb_p[:],
                        in_offset=bass.IndirectOffsetOnAxis(ap=cu[:, j : j + 1], axis=0),
                    )
                for j in range(NCAND):
                    nc.vector.tensor_tensor(prod3[:, j], wrow3[:, j], h1a[:], op=OP.mult)
                nc.vector.tensor_reduce(pk[:, 0:NCAND], prod3[:], axis=AX.X, op=OP.add)

                # ---- AllGather of (3 exact vals | 3 ids) per core ----
                nc.vector.transpose(pkT[:], pk[:])
                agin = dp.tile([2 * NCAND, B], f32, tag="agin")
                nc.sync.dma_start(out=agin[:, 0:32], in_=pkT[0 : 2 * NCAND, 0:32])
                nc.scalar.dma_start(out=agin[:, 32:64], in_=pkT[32 : 32 + 2 * NCAND, 0:32])
                agout = dp.tile([NCORES * 2 * NCAND, B], f32, tag="agout", addr_space="Shared")
                nc.gpsimd.collective_compute(
                    "AllGather", OP.bypass, replica_groups=rg,
                    ins=[agin[:].opt()], outs=[agout[:].opt()],
                )
                # block-swapped readback so one DVE stream-transpose finishes it
                nc.sync.dma_start(out=gpre[0:32, 0:32], in_=agout[0:32, 0:32])
                nc.scalar.dma_start(out=gpre[32:64, 0:32], in_=agout[0:32, 32:64])
                nc.vector.transpose(gpT[:], gpre[:])
                gall = gpT[:, 0:32].rearrange("b (r s) -> b r s", s=2 * NCAND)
                gv = gall[:, :, 0:NCAND]
                gi = gall[:, :, NCAND : 2 * NCAND]

                # ---- global argmax over 24 exact candidates (min-id tiebreak) ----
                nc.vector.max(out=gm8[:], in_=gv)
                nc.vector.tensor_scalar(msk[:], gv, gm8[:, 0:1], None, op0=OP.is_ge)
                nc.vector.tensor_scalar(msk[:], msk[:], -BIG, BIG, op0=OP.mult, op1=OP.add)
                nc.vector.tensor_tensor(msk[:], gi, msk[:], op=OP.add)
                nc.vector.tensor_reduce(gidxf[:], msk[:], axis=AX.XY, op=OP.min)
                nc.vector.tensor_copy(gidx[:], gidxf[:])

                # ---- gather the token's precomputed gate row ----
                nc.gpsimd.indirect_dma_start(
                    out=gx[:], out_offset=None, in_=whe_p[:],
                    in_offset=bass.IndirectOffsetOnAxis(ap=gidx[:, :1], axis=0),
                )

    nc.finalize()  # Bacc: runs compile() legalization passes
    return nc


# gate-unit permutation [i, f, o, g] (torch order in the weights is i, f, g, o)
_PERM = np.concatenate([np.arange(0, 1024), np.arange(1536, 2048), np.arange(1024, 1536)])


def make_in_maps(inputs):
    inp = {k: np.asarray(v) for k, v in inputs.items()}
    h0 = inp["h0"].astype(np.float32)
    c0 = inp["c0"].astype(np.float32)
    W_ih = inp["W_ih"].astype(np.float32)
    W_hh = inp["W_hh"].astype(np.float32)
    b = (inp["b_ih"].astype(np.float32) + inp["b_hh"].astype(np.float32))
    W_out = inp["W_out"].astype(np.float32)
    b_out = inp["b_out"].astype(np.float32)
    emb = inp["embed_table"].astype(np.float32)
    # x @ W_ih.T + b for every vocab row, fp32, gate units permuted to [i,f,o,g]
    whe = np.ascontiguousarray((emb @ W_ih.T + b)[:, _PERM].astype(np.float32))
    wfullb = np.ascontiguousarray(
        np.concatenate([W_out, b_out.reshape(V, 1)], axis=1).astype(np.float32))
    whhT = np.ascontiguousarray(W_hh[_PERM].T)
    bias = np.ascontiguousarray(b[_PERM].reshape(1, G4))
    in_maps = []
    for c in range(NCORES):
        base = c * VS
        tbase = np.zeros((B, NT * TPT), np.float32)
        for n in range(NT):
            tbase[:, TPT * n : TPT * (n + 1)] = float(base + n * TN)
        in_maps.append({
            "h0T": np.ascontiguousarray(h0.T),
            "c0": np.ascontiguousarray(c0),
            "whhT": whhT,
            "bias": bias,
            "woutT": np.ascontiguousarray(W_out[base : base + VS].T),
            "bout": np.ascontiguousarray(b_out[base : base + VS].reshape(1, VS)),
            "whe": whe,
            "wfullb": wfullb,
            "tbase": tbase,
        })
    return in_maps


def run(inputs, steps=S, trace=False):
    from concourse.bass_utils import run_bass_kernel_spmd

    nc = build_program(steps)
    res = run_bass_kernel_spmd(nc, make_in_maps(inputs), list(range(NCORES)),
                               trace=trace)
    outs = [res.results[c]["out"] for c in range(NCORES)]      # each [steps, B, VS]
    full = np.concatenate(outs, axis=2)                        # [steps, B, V]
    return np.ascontiguousarray(np.transpose(full, (1, 0, 2))), res


def kernel(**inputs):
    out, _ = run(inputs, steps=S, trace=False)
    return out.astype(np.float32)
